# revision 16
# baseline (speedup 1.0000x reference)
"""Trainium2 Bass kernel for nn_CIGTLayer (moe_routing).

Computation (per reference):
  xd  = fp8_e4m3fn quantize-dequant roundtrip of x (block=128 along D)
  ikd = roundtrip(info_kernel), pkd = roundtrip(path_kernels)   [host-side: tiny]
  info_gains   = sigmoid(xd @ ikd + info_bias)                  -> cumul_info output
  path_weights = sigmoid((info_gains - 0.1) * 10)
  out = (einsum('nd,pdf->npf', xd, pkd) + path_biases) * path_weights[:, :, None]

Sharding: data-parallel over tokens N across 8 cores; weights replicated.

On-chip numerics:
  - The reference's e4m3fn (max 448) roundtrip of x is reproduced with the TRN
    e4m3 (max 240) cast by doubling the scale: the e4m3fn grid on [-448, 448]
    equals 2x the TRN e4m3 grid on [-224, 224] (exact for normals; subnormal
    zone differs by <= 2^-9 * scale, negligible).
  - The GEMM runs on the PE in fp16 (operands exact in the PE's e10m11
    internal format); only the fp16 rounding of the dequantized values is an
    approximation (~3e-4 L2 relative error on the final output).
  - x arrives [n, d] but the PE contracts along partitions, so quantized+
    dequantized fp16 tiles are transposed [128,128] via DMA-transpose (XBAR).
"""

import numpy as np
import ml_dtypes

N, D, P, Fp = 32768, 1024, 4, 256
F = P * Fp                 # 1024 path output cols
FW = F + P                 # + 4 info cols
NCORES = 8
NS = N // NCORES           # 4096 tokens per core
TOK = 128                  # token tile (PSUM partitions)
NT = NS // TOK             # 32 tiles per core
NB = D // 128              # 8 contraction blocks

_cache = {}


def _quantize_roundtrip_np(a, block=128):
    b = min(block, a.shape[-1])
    ab = a.reshape(*a.shape[:-1], a.shape[-1] // b, b)
    scale = np.abs(ab).max(axis=-1, keepdims=True) / np.float32(448.0)
    scale = np.maximum(scale, np.float32(1e-12)).astype(np.float32)
    q = (ab / scale).astype(ml_dtypes.float8_e4m3fn)
    return (q.astype(np.float32) * scale).reshape(a.shape).astype(np.float32)


def build_nc(has_bias: bool):
    import concourse.mybir as mybir
    import concourse.tile as tile
    from concourse import bacc
    from contextlib import ExitStack

    AF = mybir.ActivationFunctionType
    ALU = mybir.AluOpType
    dt = mybir.dt

    nc = bacc.Bacc("TRN2", target_bir_lowering=False, debug=False,
                   num_devices=NCORES)
    x_d = nc.dram_tensor("x", [NS, D], dt.float32, kind="ExternalInput").ap()
    w_d = nc.dram_tensor("w", [D, FW], dt.float16, kind="ExternalInput").ap()
    b_d = nc.dram_tensor("b", [1, FW], dt.float16, kind="ExternalInput").ap()
    out_d = nc.dram_tensor("out", [NS, F], dt.float32, kind="ExternalOutput").ap()
    ci_d = nc.dram_tensor("ci", [NS, P], dt.float32, kind="ExternalOutput").ap()

    with tile.TileContext(nc) as tc, ExitStack() as ctx:
        consts = ctx.enter_context(tc.tile_pool(name="consts", bufs=1))
        xpool = ctx.enter_context(tc.tile_pool(name="xp", bufs=8))
        qpool = ctx.enter_context(tc.tile_pool(name="qp", bufs=5))
        spool = ctx.enter_context(tc.tile_pool(name="sp", bufs=6))
        opool = ctx.enter_context(tc.tile_pool(name="op", bufs=3))
        pspool = ctx.enter_context(tc.tile_pool(name="ps", bufs=2, space="PSUM"))
        psipool = ctx.enter_context(tc.tile_pool(name="psi", bufs=2, space="PSUM"))

        w_sb = consts.tile([128, NB, FW], dt.float16)
        for b in range(NB):
            nc.sync.dma_start(w_sb[:, b, :], w_d[b * 128:(b + 1) * 128, :])
        negone = consts.tile([TOK, 1], dt.float32)
        nc.vector.memset(negone[:], -1.0)
        if has_bias:
            bias_sb = consts.tile([1, FW], dt.float16)
            nc.sync.dma_start(bias_sb[:], b_d[:])
            ones_sb = consts.tile([1, TOK], dt.float16)
            nc.vector.memset(ones_sb[:], 1.0)

        xts = {}

        def load_tile(j):
            t = xpool.tile([TOK, NB, 128], dt.float32, tag="xt")
            nc.gpsimd.dma_start(t[:], x_d[j * TOK:(j + 1) * TOK, :])
            xts[j] = t

        PREFETCH = 6
        for j in range(min(PREFETCH, NT)):
            load_tile(j)

        for i in range(NT):
            tok = slice(i * TOK, (i + 1) * TOK)
            if i + PREFETCH < NT:
                load_tile(i + PREFETCH)
            xt = xts.pop(i)

            am = spool.tile([TOK, NB], dt.float32, tag="am")
            nc.vector.tensor_reduce(am[:], xt[:], axis=mybir.AxisListType.X,
                                    op=ALU.max, apply_absolute_value=True)
            # s2 = 2 * reference scale = max(absmax/224, 2e-12)
            s2 = spool.tile([TOK, NB], dt.float32, tag="s2")
            nc.vector.tensor_scalar(s2[:], am[:], float(np.float32(1.0 / 224.0)),
                                    2e-12, op0=ALU.mult, op1=ALU.max)
            r = spool.tile([TOK, NB], dt.float32, tag="r")
            nc.vector.reciprocal(r[:], s2[:])

            xq = qpool.tile([TOK, NB, 128], dt.float8e4, tag="xq")
            nc.vector.tensor_tensor(
                xq[:], xt[:], r[:, :, None].broadcast_to((TOK, NB, 128)),
                op=ALU.mult)
            xd = qpool.tile([TOK, NB, 128], dt.float16, tag="xd")
            nc.vector.tensor_tensor(
                xd[:], xq[:], s2[:, :, None].broadcast_to((TOK, NB, 128)),
                op=ALU.mult)
            xdT = qpool.tile([128, NB, TOK], dt.float16, tag="xdT")
            nc.sync.dma_start(xdT[:], xd[:], transpose=True)

            ps0 = pspool.tile([TOK, 512], dt.float32, tag="ps0")
            ps1 = pspool.tile([TOK, 512], dt.float32, tag="ps1")
            psI = psipool.tile([TOK, P], dt.float32, tag="psI")
            last = NB - 1
            for b in range(NB):
                st = b == 0
                sp = (b == last) and not has_bias
                nc.tensor.matmul(ps0[:], xdT[:, b, :], w_sb[:, b, 0:512],
                                 start=st, stop=sp)
                nc.tensor.matmul(ps1[:], xdT[:, b, :], w_sb[:, b, 512:1024],
                                 start=st, stop=sp)
                nc.tensor.matmul(psI[:], xdT[:, b, :], w_sb[:, b, 1024:FW],
                                 start=st, stop=sp)
            if has_bias:
                nc.tensor.matmul(ps0[:], ones_sb[:], bias_sb[:, 0:512],
                                 start=False, stop=True)
                nc.tensor.matmul(ps1[:], ones_sb[:], bias_sb[:, 512:1024],
                                 start=False, stop=True)
                nc.tensor.matmul(psI[:], ones_sb[:], bias_sb[:, 1024:FW],
                                 start=False, stop=True)

            gains = spool.tile([TOK, P], dt.float32, tag="g")
            nc.scalar.activation(gains[:], psI[:], AF.Sigmoid)
            pw = spool.tile([TOK, P], dt.float32, tag="pw")
            # sigmoid((g - 0.1) * 10) == sigmoid(10*g - 1)
            nc.scalar.activation(pw[:], gains[:], AF.Sigmoid,
                                 scale=10.0, bias=negone[:])
            nc.gpsimd.dma_start(ci_d[tok, :], gains[:])

            ot = opool.tile([TOK, F], dt.float32, tag="ot")
            for p in range(P):
                src = ps0 if p < 2 else ps1
                sl = slice((p % 2) * 256, (p % 2) * 256 + 256)
                dst = ot[:, p * 256:(p + 1) * 256]
                nc.scalar.activation(dst, src[:, sl], AF.Copy,
                                     scale=pw[:, p:p + 1])
            nc.gpsimd.dma_start(out_d[tok, :], ot[:])

    nc.compile()
    return nc


def _host_prep(info_kernel, info_bias, path_kernels, path_biases):
    ikd = _quantize_roundtrip_np(np.asarray(info_kernel, np.float32))
    pkd = _quantize_roundtrip_np(np.asarray(path_kernels, np.float32))
    w = np.empty((D, FW), np.float32)
    w[:, :F] = np.transpose(pkd, (1, 0, 2)).reshape(D, F)
    w[:, F:] = ikd
    bias = np.empty((1, FW), np.float32)
    bias[0, :F] = np.asarray(path_biases, np.float32).reshape(F)
    bias[0, F:] = np.asarray(info_bias, np.float32)
    return w.astype(np.float16), bias.astype(np.float16)


def kernel(x, info_kernel, info_bias, path_kernels, path_biases):
    from concourse.bass_utils import run_bass_kernel_spmd

    x = np.ascontiguousarray(np.asarray(x, np.float32))
    w16, bias16 = _host_prep(info_kernel, info_bias, path_kernels, path_biases)
    has_bias = bool(np.any(bias16))

    key = ("nc", has_bias)
    if key not in _cache:
        _cache[key] = build_nc(has_bias)
    nc = _cache[key]

    in_maps = [
        {"x": x[c * NS:(c + 1) * NS], "w": w16, "b": bias16}
        for c in range(NCORES)
    ]
    res = run_bass_kernel_spmd(nc, in_maps, list(range(NCORES)),
                               trace=_cache.get("trace", False))
    _cache["last_results"] = res
    out = np.concatenate([res.results[c]["out"] for c in range(NCORES)], axis=0)
    ci = np.concatenate([res.results[c]["ci"] for c in range(NCORES)], axis=0)
    return out, ci


# revision 22
# speedup vs baseline: 1.6105x; 1.6105x over previous
"""Trainium2 Bass kernel for nn_CIGTLayer (moe_routing).

Computation (per reference):
  xd  = fp8_e4m3fn quantize-dequant roundtrip of x (block=128 along D)
  ikd = roundtrip(info_kernel), pkd = roundtrip(path_kernels)   [host-side: tiny]
  info_gains   = sigmoid(xd @ ikd + info_bias)                  -> cumul_info output
  path_weights = sigmoid((info_gains - 0.1) * 10)
  out = (einsum('nd,pdf->npf', xd, pkd) + path_biases) * path_weights[:, :, None]

Sharding: data-parallel over tokens N across 8 cores; weights replicated.

On-chip numerics:
  - The reference's e4m3fn (max 448) roundtrip of x is reproduced with the TRN
    e4m3 (max 240) cast by doubling the scale: the e4m3fn grid on [-448, 448]
    equals 2x the TRN e4m3 grid on [-224, 224] (exact for normals; subnormal
    zone differs by <= 2^-9 * scale, negligible).
  - The GEMM runs on the PE in fp16 (operands exact in the PE's e10m11
    internal format); only the fp16 rounding of the dequantized values is an
    approximation (~3e-4 L2 relative error on the final output).
  - x arrives [n, d] but the PE contracts along partitions, so quantized+
    dequantized fp16 tiles are transposed [128,128] via DMA-transpose (XBAR).
"""

import numpy as np
import ml_dtypes

N, D, P, Fp = 32768, 1024, 4, 256
F = P * Fp                 # 1024 path output cols
FW = F + P                 # + 4 info cols
NCORES = 8
NS = N // NCORES           # 4096 tokens per core
TOK = 128                  # token tile (PSUM partitions)
NT = NS // TOK             # 32 tiles per core
NB = D // 128              # 8 contraction blocks

_cache = {}


def _quantize_roundtrip_np(a, block=128):
    b = min(block, a.shape[-1])
    ab = a.reshape(*a.shape[:-1], a.shape[-1] // b, b)
    scale = np.abs(ab).max(axis=-1, keepdims=True) / np.float32(448.0)
    scale = np.maximum(scale, np.float32(1e-12)).astype(np.float32)
    q = (ab / scale).astype(ml_dtypes.float8_e4m3fn)
    return (q.astype(np.float32) * scale).reshape(a.shape).astype(np.float32)


def build_nc(has_bias: bool):
    import concourse.mybir as mybir
    import concourse.tile as tile
    from concourse import bacc
    from contextlib import ExitStack

    AF = mybir.ActivationFunctionType
    ALU = mybir.AluOpType
    dt = mybir.dt

    nc = bacc.Bacc("TRN2", target_bir_lowering=False, debug=False,
                   num_devices=NCORES)
    x_d = nc.dram_tensor("x", [NS, D], dt.float32, kind="ExternalInput").ap()
    w_d = nc.dram_tensor("w", [D, FW], dt.float16, kind="ExternalInput").ap()
    b_d = nc.dram_tensor("b", [1, FW], dt.float16, kind="ExternalInput").ap()
    out_d = nc.dram_tensor("out", [NS, F], dt.float32, kind="ExternalOutput").ap()
    ci_d = nc.dram_tensor("ci", [NS, P], dt.float32, kind="ExternalOutput").ap()

    with tile.TileContext(nc) as tc, ExitStack() as ctx:
        consts = ctx.enter_context(tc.tile_pool(name="consts", bufs=1))
        xpool = ctx.enter_context(tc.tile_pool(name="xp", bufs=3))
        qpool = ctx.enter_context(tc.tile_pool(name="qp", bufs=3))
        spool = ctx.enter_context(tc.tile_pool(name="sp", bufs=6))
        opool = ctx.enter_context(tc.tile_pool(name="op", bufs=3))
        pspool = ctx.enter_context(tc.tile_pool(name="ps", bufs=3, space="PSUM"))
        psipool = ctx.enter_context(tc.tile_pool(name="psi", bufs=2, space="PSUM"))

        w_sb = consts.tile([128, NB, FW], dt.float16)
        for b in range(NB):
            nc.sync.dma_start(w_sb[:, b, :], w_d[b * 128:(b + 1) * 128, :])
        negone = consts.tile([TOK, 1], dt.float32)
        nc.vector.memset(negone[:], -1.0)
        if has_bias:
            bias_sb = consts.tile([1, FW], dt.float16)
            nc.sync.dma_start(bias_sb[:], b_d[:])
            ones_sb = consts.tile([1, TOK], dt.float16)
            nc.vector.memset(ones_sb[:], 1.0)

        # Producer chain batched in groups of G token-tiles: one load /
        # reduce / quant / dequant / transpose op per G*TOK tokens.
        G = 4
        NG = NT // G
        grp = {}

        def produce_group(g):
            """Emit the producer chain for group g."""
            rows = x_d[g * G * TOK:(g + 1) * G * TOK, :]
            xt = xpool.tile([TOK, G, NB, 128], dt.float32, tag="xt")
            # partition p = token within tile, free (i, b, d)
            nc.sync.dma_start(
                xt[:], rows.rearrange("(i p) d -> p i d", p=TOK))
            am = spool.tile([TOK, G, NB], dt.float32, tag="am")
            nc.vector.tensor_reduce(am[:], xt[:], axis=mybir.AxisListType.X,
                                    op=ALU.max, apply_absolute_value=True)
            s2 = spool.tile([TOK, G, NB], dt.float32, tag="s2")
            nc.vector.tensor_scalar(s2[:], am[:], float(np.float32(1.0 / 224.0)),
                                    2e-12, op0=ALU.mult, op1=ALU.max)
            r = spool.tile([TOK, G, NB], dt.float32, tag="r")
            nc.vector.reciprocal(r[:], s2[:])
            xq = qpool.tile([TOK, G, NB, 128], dt.float8e4, tag="xq")
            nc.vector.tensor_tensor(
                xq[:], xt[:],
                r[:, :, :, None].broadcast_to((TOK, G, NB, 128)), op=ALU.mult)
            xd = qpool.tile([TOK, G, NB, 128], dt.float16, tag="xd")
            nc.gpsimd.tensor_tensor(
                xd[:], xq[:],
                s2[:, :, :, None].broadcast_to((TOK, G, NB, 128)), op=ALU.mult)
            xdT = qpool.tile([128, G, NB, TOK], dt.float16, tag="xdT")
            nc.sync.dma_start(xdT[:], xd[:], transpose=True)
            grp[g] = xdT

        PREFETCH = 2
        for g in range(min(PREFETCH, NG)):
            produce_group(g)

        for i in range(NT):
            tok = slice(i * TOK, (i + 1) * TOK)
            g, li = divmod(i, G)
            if li == 0 and g + PREFETCH < NG:
                produce_group(g + PREFETCH)
            xdT = grp[g][:, li]

            ps0 = pspool.tile([TOK, 512], dt.float32, tag="ps0")
            ps1 = pspool.tile([TOK, 512], dt.float32, tag="ps1")
            psI = psipool.tile([TOK, P], dt.float32, tag="psI")
            last = NB - 1
            for b in range(NB):
                st = b == 0
                sp = (b == last) and not has_bias
                nc.tensor.matmul(ps0[:], xdT[:, b, :], w_sb[:, b, 0:512],
                                 start=st, stop=sp)
                nc.tensor.matmul(ps1[:], xdT[:, b, :], w_sb[:, b, 512:1024],
                                 start=st, stop=sp)
                nc.tensor.matmul(psI[:], xdT[:, b, :], w_sb[:, b, 1024:FW],
                                 start=st, stop=sp)
            if has_bias:
                nc.tensor.matmul(ps0[:], ones_sb[:], bias_sb[:, 0:512],
                                 start=False, stop=True)
                nc.tensor.matmul(ps1[:], ones_sb[:], bias_sb[:, 512:1024],
                                 start=False, stop=True)
                nc.tensor.matmul(psI[:], ones_sb[:], bias_sb[:, 1024:FW],
                                 start=False, stop=True)

            gains = spool.tile([TOK, P], dt.float32, tag="g")
            nc.scalar.activation(gains[:], psI[:], AF.Sigmoid)
            pw = spool.tile([TOK, P], dt.float32, tag="pw")
            # sigmoid((g - 0.1) * 10) == sigmoid(10*g - 1)
            nc.scalar.activation(pw[:], gains[:], AF.Sigmoid,
                                 scale=10.0, bias=negone[:])
            nc.gpsimd.dma_start(ci_d[tok, :], gains[:])

            ot = opool.tile([TOK, F], dt.float32, tag="ot")
            for p in range(P):
                src = ps0 if p < 2 else ps1
                sl = slice((p % 2) * 256, (p % 2) * 256 + 256)
                dst = ot[:, p * 256:(p + 1) * 256]
                if p % 2 == 0:
                    nc.scalar.activation(dst, src[:, sl], AF.Copy,
                                         scale=pw[:, p:p + 1])
                else:
                    nc.vector.tensor_scalar_mul(dst, src[:, sl],
                                                pw[:, p:p + 1])
            nc.scalar.dma_start(out_d[tok, :], ot[:])

    nc.compile()
    return nc


def _host_prep(info_kernel, info_bias, path_kernels, path_biases):
    ikd = _quantize_roundtrip_np(np.asarray(info_kernel, np.float32))
    pkd = _quantize_roundtrip_np(np.asarray(path_kernels, np.float32))
    w = np.empty((D, FW), np.float32)
    w[:, :F] = np.transpose(pkd, (1, 0, 2)).reshape(D, F)
    w[:, F:] = ikd
    bias = np.empty((1, FW), np.float32)
    bias[0, :F] = np.asarray(path_biases, np.float32).reshape(F)
    bias[0, F:] = np.asarray(info_bias, np.float32)
    return w.astype(np.float16), bias.astype(np.float16)


def kernel(x, info_kernel, info_bias, path_kernels, path_biases):
    from concourse.bass_utils import run_bass_kernel_spmd

    x = np.ascontiguousarray(np.asarray(x, np.float32))
    w16, bias16 = _host_prep(info_kernel, info_bias, path_kernels, path_biases)
    has_bias = bool(np.any(bias16))

    key = ("nc", has_bias)
    if key not in _cache:
        _cache[key] = build_nc(has_bias)
    nc = _cache[key]

    in_maps = [
        {"x": x[c * NS:(c + 1) * NS], "w": w16, "b": bias16}
        for c in range(NCORES)
    ]
    res = run_bass_kernel_spmd(nc, in_maps, list(range(NCORES)),
                               trace=_cache.get("trace", False))
    _cache["last_results"] = res
    out = np.concatenate([res.results[c]["out"] for c in range(NCORES)], axis=0)
    ci = np.concatenate([res.results[c]["ci"] for c in range(NCORES)], axis=0)
    return out, ci
